# revision 14
# baseline (speedup 1.0000x reference)
"""LocalizedFiltering (conv1->conv2->residual->RMSNorm) TRN2 Bass kernel.

Full inputs in, full outputs out. Internally: data-parallel over 8 NeuronCores,
2048 tokens per core (each of the 4 sequences of 4096 tokens is split in half;
even cores take sequence starts, odd cores the second halves).

Device layout is channel-major (tokens on the free dim), so matmul contraction
(channels) sits on partitions for both operands. Host transposes per-core input
slabs and output slabs. Matmuls run as float32r (TF32-class rounding, full PE
rate at N=512). The causal kernel-size-2 convs need one previous token (x) and
one previous conv1 output (o1) per shard: previous-x rows come in via the input
slab; previous-o1 is computed on device from the two extra x rows (N=2 matmuls
folded into the conv1 weight loop) and blended against the lf2 cache with a
per-core 0/1 scalar so sequence-start cores use the cache instead. RMSNorm's
cross-partition sum uses a ones-matmul (bf16) accumulating into PSUM, which
also replicates the per-token sum across all partitions for the final scale.
"""

import numpy as np
from contextlib import ExitStack

NCORES = 8
B, S, D = 4, 4096, 2048
DH = D // 2
T = (B * S) // NCORES  # tokens per core
EPS = 1e-6


# ---------------------------------------------------------------- device code


def build_module(D_, DH_, T_, NCH, EPS_CONST=EPS):
    """Build + compile the per-core Bass module. All dims in channel units;
    NCH = token chunk width for matmuls (free dim)."""
    import concourse.tile as tile
    from concourse import bacc, mybir

    f32 = mybir.dt.float32
    f32r = mybir.dt.float32r
    bf16 = mybir.dt.bfloat16
    ADD = mybir.AluOpType.add
    MUL = mybir.AluOpType.mult

    nD = D_ // 128   # input-channel tiles (16)
    nE = DH_ // 128  # hidden-channel tiles (8)
    H = T_ // 2      # half size (1024)
    NCL = H // NCH   # chunks per half (2)

    nc = bacc.Bacc("TRN2", target_bir_lowering=False, debug=False)

    xT = nc.dram_tensor("xT", [D_, T_ + 2], f32r, kind="ExternalInput")
    # weights pre-packed on host, lhsT tile-major:
    # w1pk[e, tap, d, p, m] = w1[e*128+m, d*128+p, tap]
    w1pk = nc.dram_tensor("w1pk", [nE, 2, nD, 128, 128], f32r, kind="ExternalInput")
    # w2pk[do, tap, e, p, m] = w2[do*128+m, e*128+p, tap]
    w2pk = nc.dram_tensor("w2pk", [nD, 2, nE, 128, 128], f32r, kind="ExternalInput")
    b1v = nc.dram_tensor("b1v", [DH_], f32, kind="ExternalInput")
    b2v = nc.dram_tensor("b2v", [D_], f32, kind="ExternalInput")
    lnwv = nc.dram_tensor("lnwv", [D_], f32, kind="ExternalInput")
    # aux[:, e] = o1 state for token t0-1, hidden tile e (start cores: lf2
    # cache; mid cores: host-computed single-token conv1)
    aux = nc.dram_tensor("aux", [128, nE], f32, kind="ExternalInput")

    outT = nc.dram_tensor("outT", [D_, T_], f32, kind="ExternalOutput")
    o1last = nc.dram_tensor("o1last", [DH_], f32, kind="ExternalOutput")

    with tile.TileContext(nc) as tc:
        with ExitStack() as ctx:
            const = ctx.enter_context(tc.tile_pool(name="const", bufs=1))
            o1p = ctx.enter_context(tc.tile_pool(name="o1p", bufs=1))
            xhp = ctx.enter_context(tc.tile_pool(name="xhp", bufs=1))
            w1p = ctx.enter_context(tc.tile_pool(name="w1p", bufs=4))
            w2p = ctx.enter_context(tc.tile_pool(name="w2p", bufs=8))
            yp = ctx.enter_context(tc.tile_pool(name="yp", bufs=1))
            stp = ctx.enter_context(tc.tile_pool(name="stp", bufs=1))
            invp = ctx.enter_context(tc.tile_pool(name="invp", bufs=2))
            ysqp = ctx.enter_context(tc.tile_pool(name="ysqp", bufs=1))
            sqp = ctx.enter_context(tc.tile_pool(name="sqp", bufs=1))
            outp = ctx.enter_context(tc.tile_pool(name="outp", bufs=2))
            ps_o1 = ctx.enter_context(tc.tile_pool(name="ps_o1", bufs=2, space="PSUM"))
            ps_y = ctx.enter_context(tc.tile_pool(name="ps_y", bufs=4, space="PSUM"))
            ps_ssq = ctx.enter_context(tc.tile_pool(name="ps_ssq", bufs=2, space="PSUM"))

            # constants
            b1sb = const.tile([128, nE], f32, tag="b1sb")
            b2sb = const.tile([128, nD], f32, tag="b2sb")
            lnwsb = const.tile([128, nD], f32, tag="lnwsb")
            auxsb = const.tile([128, nE], f32, tag="auxsb")
            ones = const.tile([128, 128], f32, tag="ones")
            epssb = const.tile([128, 1], f32, tag="epssb")
            nc.vector.memset(epssb[:], EPS_CONST)
            nc.sync.dma_start(out=b1sb[:], in_=b1v.ap().rearrange("(e p) -> p e", p=128))
            nc.sync.dma_start(out=b2sb[:], in_=b2v.ap().rearrange("(e p) -> p e", p=128))
            nc.sync.dma_start(out=lnwsb[:], in_=lnwv.ap().rearrange("(e p) -> p e", p=128))
            nc.sync.dma_start(out=auxsb[:], in_=aux.ap())
            nc.vector.memset(ones[:], 1.0)

            # conv1 output for the current half, channel-major [DH_, H+1];
            # col 0 = previous token's o1 (blend for h0, chained from col H after)
            o1T = [o1p.tile([128, H + 1], f32r, tag=f"o1_{e}", name=f"o1_{e}")
                   for e in range(nE)]

            # previous-token o1 state into col 0 of each o1T tile
            for e in range(nE):
                nc.vector.tensor_copy(o1T[e][:, 0:1], auxsb[:, e:e + 1])

            # ~3.4us of dummy matmuls so PE_HAM un-throttles before the DMAs
            # land and real matmuls begin (f32: 4 cycles/row)
            warm = ps_ssq.tile([128, NCH], f32, tag="ps_ssq", name="warm")
            wN = min(128, NCH)
            for i in range(8):
                nc.tensor.matmul(warm[:, 0:wN], ones[:], ones[:, 0:wN],
                                 start=(i == 0), stop=(i == 7))

            for h in range(2):
                # ---- load x half: cols [h*H, h*H + H + 2) of xT
                xh = []
                for d in range(nD):
                    t = xhp.tile([128, H + 2], f32r, tag=f"xh_{d}", name=f"xh_{d}")
                    nc.sync.dma_start(
                        out=t[:], in_=xT.ap()[d * 128:(d + 1) * 128, h * H: h * H + H + 2]
                    )
                    xh.append(t)

                # ---- conv1: o1T local cols [1 + cl*NCH, +NCH) per chunk
                for e in range(nE):
                    wb = []
                    for tap in (0, 1):
                        w = w1p.tile([128, nD, 128], f32r, tag="w1b", name="w1b")
                        nc.gpsimd.dma_start(
                            out=w[:], in_=w1pk.ap()[e, tap].rearrange("d p m -> p d m")
                        )
                        wb.append(w)
                    pss = [ps_o1.tile([128, NCH], f32, tag="ps_o1", name="ps_o1")
                           for _ in range(NCL)]
                    first = True
                    for d in range(nD):
                        for tap in (0, 1):
                            wt = wb[tap][:, d, :]
                            for cl in range(NCL):
                                k0 = cl * NCH + 1 + tap
                                nc.tensor.matmul(
                                    pss[cl][:], wt, xh[d][:, k0:k0 + NCH],
                                    start=first, stop=(d == nD - 1 and tap == 1),
                                )
                            first = False
                    for cl in range(NCL):
                        nc.vector.tensor_scalar_add(
                            o1T[e][:, 1 + cl * NCH:1 + (cl + 1) * NCH], pss[cl][:],
                            b1sb[:, e:e + 1],
                        )

                # ---- conv2 + residual + RMSNorm per chunk
                # all w2 loads for the half emitted first: keeps the Scalar
                # DMA queue free of compute-dependent ops ahead of them
                wb2s = []
                for cl in range(NCL):
                    per_cl = []
                    for dout in range(nD):
                        per_tap = []
                        for tap in (0, 1):
                            w = w2p.tile([128, nE, 128], f32r, tag="w2b", name="w2b")
                            nc.scalar.dma_start(
                                out=w[:],
                                in_=w2pk.ap()[dout, tap].rearrange("e p m -> p e m"),
                            )
                            per_tap.append(w)
                        per_cl.append(per_tap)
                    wb2s.append(per_cl)
                for cl in range(NCL):
                    J0g = h * H + cl * NCH   # global output col
                    J0 = cl * NCH            # local o1T col
                    pssq = ps_ssq.tile([128, NCH], f32, tag="ps_ssq", name="ps_ssq")
                    ssqacc = sqp.tile([128, NCH], f32, tag="ssqacc", name="ssqacc")
                    ys = []
                    for dout in range(nD):
                        wb2 = wb2s[cl][dout]
                        py = ps_y.tile([128, NCH], f32, tag="ps_y", name="ps_y")
                        first = True
                        for e in range(nE):
                            for tap in (0, 1):
                                nc.tensor.matmul(
                                    py[:], wb2[tap][:, e, :],
                                    o1T[e][:, J0 + tap:J0 + tap + NCH],
                                    start=first, stop=(e == nE - 1 and tap == 1),
                                )
                                first = False
                        # y = (psum + b2) + x
                        yt = yp.tile([128, NCH], f32, tag=f"y_{dout}", name=f"y_{dout}")
                        k0 = cl * NCH + 2
                        nc.vector.scalar_tensor_tensor(
                            out=yt[:], in0=py[:], scalar=b2sb[:, dout:dout + 1],
                            in1=xh[dout][:, k0:k0 + NCH].bitcast(f32),
                            op0=ADD, op1=ADD,
                        )
                        ys.append(yt)
                        if dout == 0:
                            nc.vector.tensor_mul(ssqacc[:], yt[:], yt[:])
                        else:
                            ysq = ysqp.tile([128, NCH], f32, tag="ysq", name="ysq")
                            nc.vector.tensor_mul(ysq[:], yt[:], yt[:])
                            nc.vector.tensor_add(ssqacc[:], ssqacc[:], ysq[:])
                    # cross-partition sum, replicated to all partitions (f32 MM)
                    nc.tensor.matmul(pssq[:], ones[:], ssqacc[:],
                                     start=True, stop=True)
                    # inv_rms = 1/sqrt(mean + eps), replicated on all partitions
                    st = stp.tile([128, NCH], f32, tag="st", name="st")
                    nc.scalar.activation(
                        out=st[:], in_=pssq[:],
                        func=mybir.ActivationFunctionType.Sqrt,
                        bias=epssb[:], scale=1.0 / D_,
                    )
                    inv = invp.tile([128, NCH], f32, tag="inv", name="inv")
                    nc.vector.reciprocal(inv[:], st[:])
                    for dout in range(nD):
                        ot = outp.tile([128, NCH], f32, tag="ot", name="ot")
                        nc.vector.scalar_tensor_tensor(
                            out=ot[:], in0=ys[dout][:], scalar=lnwsb[:, dout:dout + 1],
                            in1=inv[:], op0=MUL, op1=MUL,
                        )
                        nc.scalar.dma_start(
                            out=outT.ap()[dout * 128:(dout + 1) * 128, J0g:J0g + NCH],
                            in_=ot[:],
                        )

                if h == 0:
                    # chain the half boundary: o1(t0+H-1) -> col 0 for half 1
                    for e in range(nE):
                        nc.vector.tensor_copy(o1T[e][:, 0:1], o1T[e][:, H:H + 1])

            # last conv1 state (token t0+T-1) for the lf2 cache output
            for e in range(nE):
                nc.scalar.dma_start(
                    out=o1last.ap().rearrange("(e p) -> p e", p=128)[:, e:e + 1],
                    in_=o1T[e][:, H:H + 1].bitcast(f32),
                )

    nc.compile()
    return nc


# ------------------------------------------------------------------ host glue


def prepare_core_inputs(x3, lf1_cache, lf2_cache, w1, b1, w2, b2, ln_w,
                        ncores, S_, D_, DH_):
    """Build per-core in_maps. x3: [B, S, D] float32."""
    nD = D_ // 128
    nE = DH_ // 128
    B_ = x3.shape[0]
    T_ = (B_ * S_) // ncores
    per_seq = S_ // T_  # cores per sequence

    # lhsT tile-major packs (see build_module comments)
    w1pk = np.ascontiguousarray(
        w1.reshape(nE, 128, nD, 128, 2).transpose(0, 4, 2, 3, 1).astype(np.float32)
    )
    w2pk = np.ascontiguousarray(
        w2.reshape(nD, 128, nE, 128, 2).transpose(0, 4, 2, 3, 1).astype(np.float32)
    )
    b1c = np.ascontiguousarray(b1, np.float32)
    b2c = np.ascontiguousarray(b2, np.float32)
    lnc = np.ascontiguousarray(ln_w, np.float32)

    in_maps = []
    for c in range(ncores):
        b = c // per_seq
        part = c % per_seq
        t0 = part * T_
        x_ext = np.empty((T_ + 2, D_), np.float32)
        if part == 0:
            x_ext[0] = 0.0
            x_ext[1] = lf1_cache[b, :, 0, 0]
            o1_prev = lf2_cache[b, :, 0, 0]
        else:
            x_ext[0] = x3[b, t0 - 2]
            x_ext[1] = x3[b, t0 - 1]
            # single-token conv1 for the shard-boundary o1 state
            o1_prev = (w1[:, :, 0].astype(np.float32) @ x_ext[0]
                       + w1[:, :, 1].astype(np.float32) @ x_ext[1]
                       + b1.astype(np.float32))
        aux = np.ascontiguousarray(o1_prev.reshape(nE, 128).T.astype(np.float32))
        x_ext[2:] = x3[b, t0:t0 + T_]
        xT = np.ascontiguousarray(x_ext.T)
        in_maps.append({
            "xT": xT, "w1pk": w1pk, "w2pk": w2pk,
            "b1v": b1c, "b2v": b2c, "lnwv": lnc, "aux": aux,
        })
    return in_maps


_CACHE = {}


def _get_module():
    key = (D, DH, T)
    if key not in _CACHE:
        _CACHE[key] = build_module(D, DH, T, 512)
    return _CACHE[key]


def kernel(inputs, lf1_cache, lf2_cache, w1, b1, w2, b2, ln_w):
    from concourse.bass_utils import run_bass_kernel_spmd

    x = np.asarray(inputs, np.float32)
    lf1_cache = np.asarray(lf1_cache, np.float32)
    lf2_cache = np.asarray(lf2_cache, np.float32)
    w1 = np.asarray(w1, np.float32)
    b1 = np.asarray(b1, np.float32)
    w2 = np.asarray(w2, np.float32)
    b2 = np.asarray(b2, np.float32)
    ln_w = np.asarray(ln_w, np.float32)

    x3 = x.reshape(B, S, D)
    in_maps = prepare_core_inputs(x3, lf1_cache, lf2_cache, w1, b1, w2, b2,
                                  ln_w, NCORES, S, D, DH)
    nc = _get_module()
    res = run_bass_kernel_spmd(nc, in_maps, core_ids=list(range(NCORES)))

    per_seq = S // T
    lf_output = np.empty((B, S, D), np.float32)
    lf2 = np.empty((B, DH, 1, 1), np.float32)
    for c in range(NCORES):
        b = c // per_seq
        part = c % per_seq
        t0 = part * T
        lf_output[b, t0:t0 + T] = res.results[c]["outT"].T
        if part == per_seq - 1:
            lf2[b, :, 0, 0] = res.results[c]["o1last"]
    lf1 = np.ascontiguousarray(x3[:, -1][:, :, None, None])
    return lf_output, lf1, lf2


# revision 15
# speedup vs baseline: 1.0230x; 1.0230x over previous
"""LocalizedFiltering (conv1->conv2->residual->RMSNorm) TRN2 Bass kernel.

Full inputs in, full outputs out. Internally: data-parallel over 8 NeuronCores,
2048 tokens per core (each of the 4 sequences of 4096 tokens is split in half;
even cores take sequence starts, odd cores the second halves).

Device layout is channel-major (tokens on the free dim), so matmul contraction
(channels) sits on partitions for both operands. Host transposes per-core input
slabs and output slabs. Matmuls run as float32r (TF32-class rounding, full PE
rate at N=512). The causal kernel-size-2 convs need one previous token (x) and
one previous conv1 output (o1) per shard: previous-x rows come in via the input
slab; previous-o1 is computed on device from the two extra x rows (N=2 matmuls
folded into the conv1 weight loop) and blended against the lf2 cache with a
per-core 0/1 scalar so sequence-start cores use the cache instead. RMSNorm's
cross-partition sum uses a ones-matmul (bf16) accumulating into PSUM, which
also replicates the per-token sum across all partitions for the final scale.
"""

import numpy as np
from contextlib import ExitStack

NCORES = 8
B, S, D = 4, 4096, 2048
DH = D // 2
T = (B * S) // NCORES  # tokens per core
EPS = 1e-6


# ---------------------------------------------------------------- device code


def build_module(D_, DH_, T_, NCH, EPS_CONST=EPS):
    """Build + compile the per-core Bass module. All dims in channel units;
    NCH = token chunk width for matmuls (free dim)."""
    import concourse.tile as tile
    from concourse import bacc, mybir

    f32 = mybir.dt.float32
    f32r = mybir.dt.float32r
    bf16 = mybir.dt.bfloat16
    ADD = mybir.AluOpType.add
    MUL = mybir.AluOpType.mult

    nD = D_ // 128   # input-channel tiles (16)
    nE = DH_ // 128  # hidden-channel tiles (8)
    H = T_ // 2      # half size (1024)
    NCL = H // NCH   # chunks per half (2)

    nc = bacc.Bacc("TRN2", target_bir_lowering=False, debug=False)

    xT = nc.dram_tensor("xT", [D_, T_ + 2], f32r, kind="ExternalInput")
    # weights pre-packed on host, lhsT tile-major:
    # w1pk[e, tap, d, p, m] = w1[e*128+m, d*128+p, tap]
    w1pk = nc.dram_tensor("w1pk", [nE, 2, nD, 128, 128], f32r, kind="ExternalInput")
    # w2pk[do, tap, e, p, m] = w2[do*128+m, e*128+p, tap]
    w2pk = nc.dram_tensor("w2pk", [nD, 2, nE, 128, 128], f32r, kind="ExternalInput")
    b1v = nc.dram_tensor("b1v", [DH_], f32, kind="ExternalInput")
    b2v = nc.dram_tensor("b2v", [D_], f32, kind="ExternalInput")
    lnwv = nc.dram_tensor("lnwv", [D_], f32, kind="ExternalInput")
    # aux[:, e] = o1 state for token t0-1, hidden tile e (start cores: lf2
    # cache; mid cores: host-computed single-token conv1)
    aux = nc.dram_tensor("aux", [128, nE], f32, kind="ExternalInput")

    outT = nc.dram_tensor("outT", [D_, T_], f32, kind="ExternalOutput")
    o1last = nc.dram_tensor("o1last", [DH_], f32, kind="ExternalOutput")

    with tile.TileContext(nc) as tc:
        with ExitStack() as ctx:
            const = ctx.enter_context(tc.tile_pool(name="const", bufs=1))
            o1p = ctx.enter_context(tc.tile_pool(name="o1p", bufs=1))
            xhp = ctx.enter_context(tc.tile_pool(name="xhp", bufs=1))
            w1p = ctx.enter_context(tc.tile_pool(name="w1p", bufs=4))
            w2p = ctx.enter_context(tc.tile_pool(name="w2p", bufs=8))
            yp = ctx.enter_context(tc.tile_pool(name="yp", bufs=1))
            stp = ctx.enter_context(tc.tile_pool(name="stp", bufs=1))
            invp = ctx.enter_context(tc.tile_pool(name="invp", bufs=2))
            ysqp = ctx.enter_context(tc.tile_pool(name="ysqp", bufs=1))
            sqp = ctx.enter_context(tc.tile_pool(name="sqp", bufs=1))
            outp = ctx.enter_context(tc.tile_pool(name="outp", bufs=2))
            ps_o1 = ctx.enter_context(tc.tile_pool(name="ps_o1", bufs=3, space="PSUM"))
            ps_y = ctx.enter_context(tc.tile_pool(name="ps_y", bufs=4, space="PSUM"))
            ps_ssq = ctx.enter_context(tc.tile_pool(name="ps_ssq", bufs=1, space="PSUM"))

            # constants
            b1sb = const.tile([128, nE], f32, tag="b1sb")
            b2sb = const.tile([128, nD], f32, tag="b2sb")
            lnwsb = const.tile([128, nD], f32, tag="lnwsb")
            auxsb = const.tile([128, nE], f32, tag="auxsb")
            ones = const.tile([128, 128], f32, tag="ones")
            epssb = const.tile([128, 1], f32, tag="epssb")
            nc.vector.memset(epssb[:], EPS_CONST)
            nc.sync.dma_start(out=b1sb[:], in_=b1v.ap().rearrange("(e p) -> p e", p=128))
            nc.sync.dma_start(out=b2sb[:], in_=b2v.ap().rearrange("(e p) -> p e", p=128))
            nc.sync.dma_start(out=lnwsb[:], in_=lnwv.ap().rearrange("(e p) -> p e", p=128))
            nc.sync.dma_start(out=auxsb[:], in_=aux.ap())
            nc.vector.memset(ones[:], 1.0)

            # conv1 output for the current half, channel-major [DH_, H+1];
            # col 0 = previous token's o1 (blend for h0, chained from col H after)
            o1T = [o1p.tile([128, H + 1], f32r, tag=f"o1_{e}", name=f"o1_{e}")
                   for e in range(nE)]

            # previous-token o1 state into col 0 of each o1T tile
            for e in range(nE):
                nc.vector.tensor_copy(o1T[e][:, 0:1], auxsb[:, e:e + 1])

            # ~3.4us of dummy matmuls so PE_HAM un-throttles before the DMAs
            # land and real matmuls begin (f32: 4 cycles/row)
            warm = ps_ssq.tile([128, NCH], f32, tag="ps_ssq", name="warm")
            wN = min(128, NCH)
            for i in range(24):
                nc.tensor.matmul(warm[:, 0:wN], ones[:], ones[:, 0:wN],
                                 start=(i == 0), stop=(i == 23))

            for h in range(2):
                # ---- load x half: cols [h*H, h*H + H + 2) of xT
                xh = []
                for d in range(nD):
                    t = xhp.tile([128, H + 2], f32r, tag=f"xh_{d}", name=f"xh_{d}")
                    nc.sync.dma_start(
                        out=t[:], in_=xT.ap()[d * 128:(d + 1) * 128, h * H: h * H + H + 2]
                    )
                    xh.append(t)

                # ---- conv1: o1T local cols [1 + cl*NCH, +NCH) per chunk
                for e in range(nE):
                    wb = []
                    for tap in (0, 1):
                        w = w1p.tile([128, nD, 128], f32r, tag="w1b", name="w1b")
                        nc.gpsimd.dma_start(
                            out=w[:], in_=w1pk.ap()[e, tap].rearrange("d p m -> p d m")
                        )
                        wb.append(w)
                    pss = [ps_o1.tile([128, NCH], f32, tag="ps_o1", name="ps_o1")
                           for _ in range(NCL)]
                    first = True
                    for d in range(nD):
                        for tap in (0, 1):
                            wt = wb[tap][:, d, :]
                            for cl in range(NCL):
                                k0 = cl * NCH + 1 + tap
                                nc.tensor.matmul(
                                    pss[cl][:], wt, xh[d][:, k0:k0 + NCH],
                                    start=first, stop=(d == nD - 1 and tap == 1),
                                )
                            first = False
                    for cl in range(NCL):
                        nc.vector.tensor_scalar_add(
                            o1T[e][:, 1 + cl * NCH:1 + (cl + 1) * NCH], pss[cl][:],
                            b1sb[:, e:e + 1],
                        )

                # ---- conv2 + residual + RMSNorm per chunk
                # all w2 loads for the half emitted first: keeps the Scalar
                # DMA queue free of compute-dependent ops ahead of them
                wb2s = []
                for cl in range(NCL):
                    per_cl = []
                    for dout in range(nD):
                        per_tap = []
                        for tap in (0, 1):
                            w = w2p.tile([128, nE, 128], f32r, tag="w2b", name="w2b")
                            nc.scalar.dma_start(
                                out=w[:],
                                in_=w2pk.ap()[dout, tap].rearrange("e p m -> p e m"),
                            )
                            per_tap.append(w)
                        per_cl.append(per_tap)
                    wb2s.append(per_cl)
                for cl in range(NCL):
                    J0g = h * H + cl * NCH   # global output col
                    J0 = cl * NCH            # local o1T col
                    pssq = ps_ssq.tile([128, NCH], f32, tag="ps_ssq", name="ps_ssq")
                    ssqacc = sqp.tile([128, NCH], f32, tag="ssqacc", name="ssqacc")
                    ys = []
                    for dout in range(nD):
                        wb2 = wb2s[cl][dout]
                        py = ps_y.tile([128, NCH], f32, tag="ps_y", name="ps_y")
                        first = True
                        for e in range(nE):
                            for tap in (0, 1):
                                nc.tensor.matmul(
                                    py[:], wb2[tap][:, e, :],
                                    o1T[e][:, J0 + tap:J0 + tap + NCH],
                                    start=first, stop=(e == nE - 1 and tap == 1),
                                )
                                first = False
                        # y = (psum + b2) + x
                        yt = yp.tile([128, NCH], f32, tag=f"y_{dout}", name=f"y_{dout}")
                        k0 = cl * NCH + 2
                        nc.vector.scalar_tensor_tensor(
                            out=yt[:], in0=py[:], scalar=b2sb[:, dout:dout + 1],
                            in1=xh[dout][:, k0:k0 + NCH].bitcast(f32),
                            op0=ADD, op1=ADD,
                        )
                        ys.append(yt)
                        if dout == 0:
                            nc.vector.tensor_mul(ssqacc[:], yt[:], yt[:])
                        else:
                            ysq = ysqp.tile([128, NCH], f32, tag="ysq", name="ysq")
                            nc.vector.tensor_mul(ysq[:], yt[:], yt[:])
                            nc.vector.tensor_add(ssqacc[:], ssqacc[:], ysq[:])
                    # cross-partition sum, replicated to all partitions (f32 MM)
                    nc.tensor.matmul(pssq[:], ones[:], ssqacc[:],
                                     start=True, stop=True)
                    # inv_rms = 1/sqrt(mean + eps), replicated on all partitions
                    st = stp.tile([128, NCH], f32, tag="st", name="st")
                    nc.scalar.activation(
                        out=st[:], in_=pssq[:],
                        func=mybir.ActivationFunctionType.Sqrt,
                        bias=epssb[:], scale=1.0 / D_,
                    )
                    inv = invp.tile([128, NCH], f32, tag="inv", name="inv")
                    nc.vector.reciprocal(inv[:], st[:])
                    for dout in range(nD):
                        ot = outp.tile([128, NCH], f32, tag="ot", name="ot")
                        nc.vector.scalar_tensor_tensor(
                            out=ot[:], in0=ys[dout][:], scalar=lnwsb[:, dout:dout + 1],
                            in1=inv[:], op0=MUL, op1=MUL,
                        )
                        # spread store issues across idle DMA queues; Sync
                        # only for h1 (its queue holds h1 xh loads that wait
                        # on h0's residual reads - HOL hazard for h0 stores)
                        if dout % 2 == 1:
                            st_eng = nc.gpsimd
                        elif h == 0:
                            st_eng = nc.scalar
                        else:
                            st_eng = nc.sync
                        st_eng.dma_start(
                            out=outT.ap()[dout * 128:(dout + 1) * 128, J0g:J0g + NCH],
                            in_=ot[:],
                        )

                if h == 0:
                    # chain the half boundary: o1(t0+H-1) -> col 0 for half 1
                    for e in range(nE):
                        nc.vector.tensor_copy(o1T[e][:, 0:1], o1T[e][:, H:H + 1])

            # last conv1 state (token t0+T-1) for the lf2 cache output
            for e in range(nE):
                nc.scalar.dma_start(
                    out=o1last.ap().rearrange("(e p) -> p e", p=128)[:, e:e + 1],
                    in_=o1T[e][:, H:H + 1].bitcast(f32),
                )

    nc.compile()
    return nc


# ------------------------------------------------------------------ host glue


def prepare_core_inputs(x3, lf1_cache, lf2_cache, w1, b1, w2, b2, ln_w,
                        ncores, S_, D_, DH_):
    """Build per-core in_maps. x3: [B, S, D] float32."""
    nD = D_ // 128
    nE = DH_ // 128
    B_ = x3.shape[0]
    T_ = (B_ * S_) // ncores
    per_seq = S_ // T_  # cores per sequence

    # lhsT tile-major packs (see build_module comments)
    w1pk = np.ascontiguousarray(
        w1.reshape(nE, 128, nD, 128, 2).transpose(0, 4, 2, 3, 1).astype(np.float32)
    )
    w2pk = np.ascontiguousarray(
        w2.reshape(nD, 128, nE, 128, 2).transpose(0, 4, 2, 3, 1).astype(np.float32)
    )
    b1c = np.ascontiguousarray(b1, np.float32)
    b2c = np.ascontiguousarray(b2, np.float32)
    lnc = np.ascontiguousarray(ln_w, np.float32)

    in_maps = []
    for c in range(ncores):
        b = c // per_seq
        part = c % per_seq
        t0 = part * T_
        x_ext = np.empty((T_ + 2, D_), np.float32)
        if part == 0:
            x_ext[0] = 0.0
            x_ext[1] = lf1_cache[b, :, 0, 0]
            o1_prev = lf2_cache[b, :, 0, 0]
        else:
            x_ext[0] = x3[b, t0 - 2]
            x_ext[1] = x3[b, t0 - 1]
            # single-token conv1 for the shard-boundary o1 state
            o1_prev = (w1[:, :, 0].astype(np.float32) @ x_ext[0]
                       + w1[:, :, 1].astype(np.float32) @ x_ext[1]
                       + b1.astype(np.float32))
        aux = np.ascontiguousarray(o1_prev.reshape(nE, 128).T.astype(np.float32))
        x_ext[2:] = x3[b, t0:t0 + T_]
        xT = np.ascontiguousarray(x_ext.T)
        in_maps.append({
            "xT": xT, "w1pk": w1pk, "w2pk": w2pk,
            "b1v": b1c, "b2v": b2c, "lnwv": lnc, "aux": aux,
        })
    return in_maps


_CACHE = {}


def _get_module():
    key = (D, DH, T)
    if key not in _CACHE:
        _CACHE[key] = build_module(D, DH, T, 512)
    return _CACHE[key]


def kernel(inputs, lf1_cache, lf2_cache, w1, b1, w2, b2, ln_w):
    from concourse.bass_utils import run_bass_kernel_spmd

    x = np.asarray(inputs, np.float32)
    lf1_cache = np.asarray(lf1_cache, np.float32)
    lf2_cache = np.asarray(lf2_cache, np.float32)
    w1 = np.asarray(w1, np.float32)
    b1 = np.asarray(b1, np.float32)
    w2 = np.asarray(w2, np.float32)
    b2 = np.asarray(b2, np.float32)
    ln_w = np.asarray(ln_w, np.float32)

    x3 = x.reshape(B, S, D)
    in_maps = prepare_core_inputs(x3, lf1_cache, lf2_cache, w1, b1, w2, b2,
                                  ln_w, NCORES, S, D, DH)
    nc = _get_module()
    res = run_bass_kernel_spmd(nc, in_maps, core_ids=list(range(NCORES)))

    per_seq = S // T
    lf_output = np.empty((B, S, D), np.float32)
    lf2 = np.empty((B, DH, 1, 1), np.float32)
    for c in range(NCORES):
        b = c // per_seq
        part = c % per_seq
        t0 = part * T
        lf_output[b, t0:t0 + T] = res.results[c]["outT"].T
        if part == per_seq - 1:
            lf2[b, :, 0, 0] = res.results[c]["o1last"]
    lf1 = np.ascontiguousarray(x3[:, -1][:, :, None, None])
    return lf_output, lf1, lf2


# revision 16
# speedup vs baseline: 1.0560x; 1.0323x over previous
"""LocalizedFiltering (conv1->conv2->residual->RMSNorm) TRN2 Bass kernel.

Full inputs in, full outputs out. Internally: data-parallel over 8 NeuronCores,
2048 tokens per core (each of the 4 sequences of 4096 tokens is split in half;
even cores take sequence starts, odd cores the second halves).

Device layout is channel-major (tokens on the free dim), so matmul contraction
(channels) sits on partitions for both operands. Host transposes per-core input
slabs and output slabs. Matmuls run as float32r (TF32-class rounding, full PE
rate at N=512). The causal kernel-size-2 convs need one previous token (x) and
one previous conv1 output (o1) per shard: previous-x rows come in via the input
slab; previous-o1 is computed on device from the two extra x rows (N=2 matmuls
folded into the conv1 weight loop) and blended against the lf2 cache with a
per-core 0/1 scalar so sequence-start cores use the cache instead. RMSNorm's
cross-partition sum uses a ones-matmul (bf16) accumulating into PSUM, which
also replicates the per-token sum across all partitions for the final scale.
"""

import numpy as np
from contextlib import ExitStack

NCORES = 8
B, S, D = 4, 4096, 2048
DH = D // 2
T = (B * S) // NCORES  # tokens per core
EPS = 1e-6


# ---------------------------------------------------------------- device code


def build_module(D_, DH_, T_, NCH, EPS_CONST=EPS):
    """Build + compile the per-core Bass module. All dims in channel units;
    NCH = token chunk width for matmuls (free dim)."""
    import concourse.tile as tile
    from concourse import bacc, mybir

    f32 = mybir.dt.float32
    f32r = mybir.dt.float32r
    bf16 = mybir.dt.bfloat16
    ADD = mybir.AluOpType.add
    MUL = mybir.AluOpType.mult

    nD = D_ // 128   # input-channel tiles (16)
    nE = DH_ // 128  # hidden-channel tiles (8)
    H = T_ // 2      # half size (1024)
    NCL = H // NCH   # chunks per half (2)

    nc = bacc.Bacc("TRN2", target_bir_lowering=False, debug=False)

    xT = nc.dram_tensor("xT", [D_, T_ + 2], f32r, kind="ExternalInput")
    # weights pre-packed on host, lhsT tile-major:
    # w1pk[e, tap, d, p, m] = w1[e*128+m, d*128+p, tap]
    w1pk = nc.dram_tensor("w1pk", [nE, 2, nD, 128, 128], f32r, kind="ExternalInput")
    # w2pk[do, tap, e, p, m] = w2[do*128+m, e*128+p, tap]
    w2pk = nc.dram_tensor("w2pk", [nD, 2, nE, 128, 128], f32r, kind="ExternalInput")
    b1v = nc.dram_tensor("b1v", [DH_], f32, kind="ExternalInput")
    b2v = nc.dram_tensor("b2v", [D_], f32, kind="ExternalInput")
    lnwv = nc.dram_tensor("lnwv", [D_], f32, kind="ExternalInput")
    # aux[:, e] = o1 state for token t0-1, hidden tile e (start cores: lf2
    # cache; mid cores: host-computed single-token conv1)
    aux = nc.dram_tensor("aux", [128, nE], f32, kind="ExternalInput")

    outT = nc.dram_tensor("outT", [D_, T_], f32, kind="ExternalOutput")
    o1last = nc.dram_tensor("o1last", [DH_], f32, kind="ExternalOutput")

    with tile.TileContext(nc) as tc:
        with ExitStack() as ctx:
            const = ctx.enter_context(tc.tile_pool(name="const", bufs=1))
            o1p = ctx.enter_context(tc.tile_pool(name="o1p", bufs=1))
            xhp = ctx.enter_context(tc.tile_pool(name="xhp", bufs=1))
            w1p = ctx.enter_context(tc.tile_pool(name="w1p", bufs=4))
            w2p = ctx.enter_context(tc.tile_pool(name="w2p", bufs=8))
            yp = ctx.enter_context(tc.tile_pool(name="yp", bufs=1))
            stp = ctx.enter_context(tc.tile_pool(name="stp", bufs=1))
            invp = ctx.enter_context(tc.tile_pool(name="invp", bufs=2))
            ysqp = ctx.enter_context(tc.tile_pool(name="ysqp", bufs=1))
            sqp = ctx.enter_context(tc.tile_pool(name="sqp", bufs=1))
            ps_o1 = ctx.enter_context(tc.tile_pool(name="ps_o1", bufs=3, space="PSUM"))
            ps_y = ctx.enter_context(tc.tile_pool(name="ps_y", bufs=4, space="PSUM"))
            ps_ssq = ctx.enter_context(tc.tile_pool(name="ps_ssq", bufs=1, space="PSUM"))

            # constants
            b1sb = const.tile([128, nE], f32, tag="b1sb")
            b2sb = const.tile([128, nD], f32, tag="b2sb")
            lnwsb = const.tile([128, nD], f32, tag="lnwsb")
            auxsb = const.tile([128, nE], f32, tag="auxsb")
            ones = const.tile([128, 128], f32, tag="ones")
            epssb = const.tile([128, 1], f32, tag="epssb")
            nc.vector.memset(epssb[:], EPS_CONST)
            nc.sync.dma_start(out=b1sb[:], in_=b1v.ap().rearrange("(e p) -> p e", p=128))
            nc.sync.dma_start(out=b2sb[:], in_=b2v.ap().rearrange("(e p) -> p e", p=128))
            nc.sync.dma_start(out=lnwsb[:], in_=lnwv.ap().rearrange("(e p) -> p e", p=128))
            nc.sync.dma_start(out=auxsb[:], in_=aux.ap())
            nc.vector.memset(ones[:], 1.0)

            # conv1 output for the current half, channel-major [DH_, H+1];
            # col 0 = previous token's o1 (blend for h0, chained from col H after)
            o1T = [o1p.tile([128, H + 1], f32r, tag=f"o1_{e}", name=f"o1_{e}")
                   for e in range(nE)]

            # previous-token o1 state into col 0 of each o1T tile
            for e in range(nE):
                nc.vector.tensor_copy(o1T[e][:, 0:1], auxsb[:, e:e + 1])

            # ~3.4us of dummy matmuls so PE_HAM un-throttles before the DMAs
            # land and real matmuls begin (f32: 4 cycles/row)
            warm = ps_ssq.tile([128, NCH], f32, tag="ps_ssq", name="warm")
            wN = min(128, NCH)
            for i in range(24):
                nc.tensor.matmul(warm[:, 0:wN], ones[:], ones[:, 0:wN],
                                 start=(i == 0), stop=(i == 23))

            for h in range(2):
                # ---- load x half: cols [h*H, h*H + H + 2) of xT
                xh = []
                for d in range(nD):
                    t = xhp.tile([128, H + 2], f32r, tag=f"xh_{d}", name=f"xh_{d}")
                    nc.sync.dma_start(
                        out=t[:], in_=xT.ap()[d * 128:(d + 1) * 128, h * H: h * H + H + 2]
                    )
                    xh.append(t)

                # ---- conv1: o1T local cols [1 + cl*NCH, +NCH) per chunk
                for e in range(nE):
                    wb = []
                    for tap in (0, 1):
                        w = w1p.tile([128, nD, 128], f32r, tag="w1b", name="w1b")
                        nc.gpsimd.dma_start(
                            out=w[:], in_=w1pk.ap()[e, tap].rearrange("d p m -> p d m")
                        )
                        wb.append(w)
                    pss = [ps_o1.tile([128, NCH], f32, tag="ps_o1", name="ps_o1")
                           for _ in range(NCL)]
                    first = True
                    for d in range(nD):
                        for tap in (0, 1):
                            wt = wb[tap][:, d, :]
                            for cl in range(NCL):
                                k0 = cl * NCH + 1 + tap
                                nc.tensor.matmul(
                                    pss[cl][:], wt, xh[d][:, k0:k0 + NCH],
                                    start=first, stop=(d == nD - 1 and tap == 1),
                                )
                            first = False
                    for cl in range(NCL):
                        nc.vector.tensor_scalar_add(
                            o1T[e][:, 1 + cl * NCH:1 + (cl + 1) * NCH], pss[cl][:],
                            b1sb[:, e:e + 1],
                        )

                # ---- conv2 + residual + RMSNorm per chunk
                # all w2 loads for the half emitted first: keeps the Scalar
                # DMA queue free of compute-dependent ops ahead of them
                wb2s = []
                for cl in range(NCL):
                    per_cl = []
                    for dout in range(nD):
                        per_tap = []
                        for tap in (0, 1):
                            w = w2p.tile([128, nE, 128], f32r, tag="w2b", name="w2b")
                            (nc.scalar if tap == 0 else nc.gpsimd).dma_start(
                                out=w[:],
                                in_=w2pk.ap()[dout, tap].rearrange("e p m -> p e m"),
                            )
                            per_tap.append(w)
                        per_cl.append(per_tap)
                    wb2s.append(per_cl)
                for cl in range(NCL):
                    J0g = h * H + cl * NCH   # global output col
                    J0 = cl * NCH            # local o1T col
                    pssq = ps_ssq.tile([128, NCH], f32, tag="ps_ssq", name="ps_ssq")
                    ssqacc = sqp.tile([128, NCH], f32, tag="ssqacc", name="ssqacc")
                    ys = []
                    for dout in range(nD):
                        wb2 = wb2s[cl][dout]
                        py = ps_y.tile([128, NCH], f32, tag="ps_y", name="ps_y")
                        first = True
                        for e in range(nE):
                            for tap in (0, 1):
                                nc.tensor.matmul(
                                    py[:], wb2[tap][:, e, :],
                                    o1T[e][:, J0 + tap:J0 + tap + NCH],
                                    start=first, stop=(e == nE - 1 and tap == 1),
                                )
                                first = False
                        # y = (psum + b2) + x
                        yt = yp.tile([128, NCH], f32, tag=f"y_{dout}", name=f"y_{dout}")
                        k0 = cl * NCH + 2
                        nc.vector.scalar_tensor_tensor(
                            out=yt[:], in0=py[:], scalar=b2sb[:, dout:dout + 1],
                            in1=xh[dout][:, k0:k0 + NCH].bitcast(f32),
                            op0=ADD, op1=ADD,
                        )
                        ys.append(yt)
                        if dout == 0:
                            nc.vector.tensor_mul(ssqacc[:], yt[:], yt[:])
                        else:
                            ysq = ysqp.tile([128, NCH], f32, tag="ysq", name="ysq")
                            nc.vector.tensor_mul(ysq[:], yt[:], yt[:])
                            nc.vector.tensor_add(ssqacc[:], ssqacc[:], ysq[:])
                    # cross-partition sum, replicated to all partitions (f32 MM)
                    nc.tensor.matmul(pssq[:], ones[:], ssqacc[:],
                                     start=True, stop=True)
                    # inv_rms = 1/sqrt(mean + eps), replicated on all partitions
                    st = stp.tile([128, NCH], f32, tag="st", name="st")
                    nc.scalar.activation(
                        out=st[:], in_=pssq[:],
                        func=mybir.ActivationFunctionType.Sqrt,
                        bias=epssb[:], scale=1.0 / D_,
                    )
                    inv = invp.tile([128, NCH], f32, tag="inv", name="inv")
                    rscr = stp.tile([128, NCH], f32, tag="rscr", name="rscr")
                    nc.vector.reciprocal_approx_accurate(inv[:], st[:], rscr[:])
                    for dout in range(nD):
                        nc.vector.scalar_tensor_tensor(
                            out=ys[dout][:], in0=ys[dout][:],
                            scalar=lnwsb[:, dout:dout + 1],
                            in1=inv[:], op0=MUL, op1=MUL,
                        )
                        # spread store issues across idle DMA queues; Sync
                        # only for h1 (its queue holds h1 xh loads that wait
                        # on h0's residual reads - HOL hazard for h0 stores)
                        if h == 0:
                            st_eng = nc.gpsimd if dout % 2 else nc.scalar
                        else:
                            st_eng = (nc.scalar, nc.gpsimd, nc.sync)[dout % 3]
                        st_eng.dma_start(
                            out=outT.ap()[dout * 128:(dout + 1) * 128, J0g:J0g + NCH],
                            in_=ys[dout][:],
                        )

                if h == 0:
                    # chain the half boundary: o1(t0+H-1) -> col 0 for half 1
                    for e in range(nE):
                        nc.vector.tensor_copy(o1T[e][:, 0:1], o1T[e][:, H:H + 1])

            # last conv1 state (token t0+T-1) for the lf2 cache output
            for e in range(nE):
                nc.scalar.dma_start(
                    out=o1last.ap().rearrange("(e p) -> p e", p=128)[:, e:e + 1],
                    in_=o1T[e][:, H:H + 1].bitcast(f32),
                )

    nc.compile()
    return nc


# ------------------------------------------------------------------ host glue


def prepare_core_inputs(x3, lf1_cache, lf2_cache, w1, b1, w2, b2, ln_w,
                        ncores, S_, D_, DH_):
    """Build per-core in_maps. x3: [B, S, D] float32."""
    nD = D_ // 128
    nE = DH_ // 128
    B_ = x3.shape[0]
    T_ = (B_ * S_) // ncores
    per_seq = S_ // T_  # cores per sequence

    # lhsT tile-major packs (see build_module comments)
    w1pk = np.ascontiguousarray(
        w1.reshape(nE, 128, nD, 128, 2).transpose(0, 4, 2, 3, 1).astype(np.float32)
    )
    w2pk = np.ascontiguousarray(
        w2.reshape(nD, 128, nE, 128, 2).transpose(0, 4, 2, 3, 1).astype(np.float32)
    )
    b1c = np.ascontiguousarray(b1, np.float32)
    b2c = np.ascontiguousarray(b2, np.float32)
    lnc = np.ascontiguousarray(ln_w, np.float32)

    in_maps = []
    for c in range(ncores):
        b = c // per_seq
        part = c % per_seq
        t0 = part * T_
        x_ext = np.empty((T_ + 2, D_), np.float32)
        if part == 0:
            x_ext[0] = 0.0
            x_ext[1] = lf1_cache[b, :, 0, 0]
            o1_prev = lf2_cache[b, :, 0, 0]
        else:
            x_ext[0] = x3[b, t0 - 2]
            x_ext[1] = x3[b, t0 - 1]
            # single-token conv1 for the shard-boundary o1 state
            o1_prev = (w1[:, :, 0].astype(np.float32) @ x_ext[0]
                       + w1[:, :, 1].astype(np.float32) @ x_ext[1]
                       + b1.astype(np.float32))
        aux = np.ascontiguousarray(o1_prev.reshape(nE, 128).T.astype(np.float32))
        x_ext[2:] = x3[b, t0:t0 + T_]
        xT = np.ascontiguousarray(x_ext.T)
        in_maps.append({
            "xT": xT, "w1pk": w1pk, "w2pk": w2pk,
            "b1v": b1c, "b2v": b2c, "lnwv": lnc, "aux": aux,
        })
    return in_maps


_CACHE = {}


def _get_module():
    key = (D, DH, T)
    if key not in _CACHE:
        _CACHE[key] = build_module(D, DH, T, 512)
    return _CACHE[key]


def kernel(inputs, lf1_cache, lf2_cache, w1, b1, w2, b2, ln_w):
    from concourse.bass_utils import run_bass_kernel_spmd

    x = np.asarray(inputs, np.float32)
    lf1_cache = np.asarray(lf1_cache, np.float32)
    lf2_cache = np.asarray(lf2_cache, np.float32)
    w1 = np.asarray(w1, np.float32)
    b1 = np.asarray(b1, np.float32)
    w2 = np.asarray(w2, np.float32)
    b2 = np.asarray(b2, np.float32)
    ln_w = np.asarray(ln_w, np.float32)

    x3 = x.reshape(B, S, D)
    in_maps = prepare_core_inputs(x3, lf1_cache, lf2_cache, w1, b1, w2, b2,
                                  ln_w, NCORES, S, D, DH)
    nc = _get_module()
    res = run_bass_kernel_spmd(nc, in_maps, core_ids=list(range(NCORES)))

    per_seq = S // T
    lf_output = np.empty((B, S, D), np.float32)
    lf2 = np.empty((B, DH, 1, 1), np.float32)
    for c in range(NCORES):
        b = c // per_seq
        part = c % per_seq
        t0 = part * T
        lf_output[b, t0:t0 + T] = res.results[c]["outT"].T
        if part == per_seq - 1:
            lf2[b, :, 0, 0] = res.results[c]["o1last"]
    lf1 = np.ascontiguousarray(x3[:, -1][:, :, None, None])
    return lf_output, lf1, lf2


# revision 17
# speedup vs baseline: 1.0635x; 1.0071x over previous
"""LocalizedFiltering (conv1->conv2->residual->RMSNorm) TRN2 Bass kernel.

Full inputs in, full outputs out. Internally: data-parallel over 8 NeuronCores,
2048 tokens per core (each of the 4 sequences of 4096 tokens is split in half;
even cores take sequence starts, odd cores the second halves).

Device layout is channel-major (tokens on the free dim), so matmul contraction
(channels) sits on partitions for both operands. Host transposes per-core input
slabs and output slabs. Matmuls run as float32r (TF32-class rounding, full PE
rate at N=512). The causal kernel-size-2 convs need one previous token (x) and
one previous conv1 output (o1) per shard: previous-x rows come in via the input
slab; previous-o1 is computed on device from the two extra x rows (N=2 matmuls
folded into the conv1 weight loop) and blended against the lf2 cache with a
per-core 0/1 scalar so sequence-start cores use the cache instead. RMSNorm's
cross-partition sum uses a ones-matmul (bf16) accumulating into PSUM, which
also replicates the per-token sum across all partitions for the final scale.
"""

import numpy as np
from contextlib import ExitStack

NCORES = 8
B, S, D = 4, 4096, 2048
DH = D // 2
T = (B * S) // NCORES  # tokens per core
EPS = 1e-6


# ---------------------------------------------------------------- device code


def build_module(D_, DH_, T_, NCH, EPS_CONST=EPS):
    """Build + compile the per-core Bass module. All dims in channel units;
    NCH = token chunk width for matmuls (free dim)."""
    import concourse.tile as tile
    from concourse import bacc, mybir

    f32 = mybir.dt.float32
    f32r = mybir.dt.float32r
    bf16 = mybir.dt.bfloat16
    ADD = mybir.AluOpType.add
    MUL = mybir.AluOpType.mult

    nD = D_ // 128   # input-channel tiles (16)
    nE = DH_ // 128  # hidden-channel tiles (8)
    H = T_ // 2      # half size (1024)
    NCL = H // NCH   # chunks per half (2)

    nc = bacc.Bacc("TRN2", target_bir_lowering=False, debug=False)

    xT = nc.dram_tensor("xT", [D_, T_ + 2], f32r, kind="ExternalInput")
    # weights pre-packed on host, lhsT tile-major:
    # w1pk[e, tap, d, p, m] = w1[e*128+m, d*128+p, tap]
    w1pk = nc.dram_tensor("w1pk", [nE, 2, nD, 128, 128], f32r, kind="ExternalInput")
    # w2pk[do, tap, e, p, m] = w2[do*128+m, e*128+p, tap]
    w2pk = nc.dram_tensor("w2pk", [nD, 2, nE, 128, 128], f32r, kind="ExternalInput")
    b1v = nc.dram_tensor("b1v", [DH_], f32, kind="ExternalInput")
    b2v = nc.dram_tensor("b2v", [D_], f32, kind="ExternalInput")
    lnwv = nc.dram_tensor("lnwv", [D_], f32, kind="ExternalInput")
    # aux[:, e] = o1 state for token t0-1, hidden tile e (start cores: lf2
    # cache; mid cores: host-computed single-token conv1)
    aux = nc.dram_tensor("aux", [128, nE], f32, kind="ExternalInput")

    outT = nc.dram_tensor("outT", [D_, T_], f32, kind="ExternalOutput")
    o1last = nc.dram_tensor("o1last", [DH_], f32, kind="ExternalOutput")

    with tile.TileContext(nc) as tc:
        with ExitStack() as ctx:
            const = ctx.enter_context(tc.tile_pool(name="const", bufs=1))
            o1p = ctx.enter_context(tc.tile_pool(name="o1p", bufs=1))
            xhp = ctx.enter_context(tc.tile_pool(name="xhp", bufs=1))
            w1p = ctx.enter_context(tc.tile_pool(name="w1p", bufs=4))
            w2p = ctx.enter_context(tc.tile_pool(name="w2p", bufs=8))
            yp = ctx.enter_context(tc.tile_pool(name="yp", bufs=1))
            stp = ctx.enter_context(tc.tile_pool(name="stp", bufs=1))
            invp = ctx.enter_context(tc.tile_pool(name="invp", bufs=2))
            ysqp = ctx.enter_context(tc.tile_pool(name="ysqp", bufs=1))
            sqp = ctx.enter_context(tc.tile_pool(name="sqp", bufs=1))
            ps_o1 = ctx.enter_context(tc.tile_pool(name="ps_o1", bufs=2, space="PSUM"))
            ps_y = ctx.enter_context(tc.tile_pool(name="ps_y", bufs=5, space="PSUM"))
            ps_ssq = ctx.enter_context(tc.tile_pool(name="ps_ssq", bufs=1, space="PSUM"))

            # constants
            b1sb = const.tile([128, nE], f32, tag="b1sb")
            b2sb = const.tile([128, nD], f32, tag="b2sb")
            lnwsb = const.tile([128, nD], f32, tag="lnwsb")
            auxsb = const.tile([128, nE], f32, tag="auxsb")
            ones = const.tile([128, 128], f32, tag="ones")
            epssb = const.tile([128, 1], f32, tag="epssb")
            nc.vector.memset(epssb[:], EPS_CONST)
            nc.sync.dma_start(out=b1sb[:], in_=b1v.ap().rearrange("(e p) -> p e", p=128))
            nc.sync.dma_start(out=b2sb[:], in_=b2v.ap().rearrange("(e p) -> p e", p=128))
            nc.sync.dma_start(out=lnwsb[:], in_=lnwv.ap().rearrange("(e p) -> p e", p=128))
            nc.sync.dma_start(out=auxsb[:], in_=aux.ap())
            nc.vector.memset(ones[:], 1.0)

            # conv1 output for the current half, channel-major [DH_, H+1];
            # col 0 = previous token's o1 (blend for h0, chained from col H after)
            o1T = [o1p.tile([128, H + 1], f32r, tag=f"o1_{e}", name=f"o1_{e}")
                   for e in range(nE)]

            # previous-token o1 state into col 0 of each o1T tile
            for e in range(nE):
                nc.vector.tensor_copy(o1T[e][:, 0:1], auxsb[:, e:e + 1])

            # ~3.4us of dummy matmuls so PE_HAM un-throttles before the DMAs
            # land and real matmuls begin (f32: 4 cycles/row)
            warm = ps_ssq.tile([128, NCH], f32, tag="ps_ssq", name="warm")
            wN = min(128, NCH)
            for i in range(24):
                nc.tensor.matmul(warm[:, 0:wN], ones[:], ones[:, 0:wN],
                                 start=(i == 0), stop=(i == 23))

            for h in range(2):
                if h == 1:
                    # keep PE busy + HAM warm across the half transition (the
                    # PE otherwise idles ~5-10us here waiting on xh reloads and
                    # re-throttles to 1.2GHz)
                    warm2 = ps_o1.tile([128, NCH], f32, tag="ps_o1", name="warm2")
                    for i in range(6):
                        nc.tensor.matmul(warm2[:, 0:wN], ones[:], ones[:, 0:wN],
                                         start=(i == 0), stop=(i == 5))

                # ---- load x half: cols [h*H, h*H + H + 2) of xT
                xh = []
                for d in range(nD):
                    t = xhp.tile([128, H + 2], f32r, tag=f"xh_{d}", name=f"xh_{d}")
                    nc.sync.dma_start(
                        out=t[:], in_=xT.ap()[d * 128:(d + 1) * 128, h * H: h * H + H + 2]
                    )
                    xh.append(t)

                # ---- conv1: o1T local cols [1 + cl*NCH, +NCH) per chunk
                for e in range(nE):
                    wb = []
                    for tap in (0, 1):
                        w = w1p.tile([128, nD, 128], f32r, tag="w1b", name="w1b")
                        nc.gpsimd.dma_start(
                            out=w[:], in_=w1pk.ap()[e, tap].rearrange("d p m -> p d m")
                        )
                        wb.append(w)
                    pss = [ps_o1.tile([128, NCH], f32, tag="ps_o1", name="ps_o1")
                           for _ in range(NCL)]
                    first = True
                    for d in range(nD):
                        for tap in (0, 1):
                            wt = wb[tap][:, d, :]
                            for cl in range(NCL):
                                k0 = cl * NCH + 1 + tap
                                nc.tensor.matmul(
                                    pss[cl][:], wt, xh[d][:, k0:k0 + NCH],
                                    start=first, stop=(d == nD - 1 and tap == 1),
                                )
                            first = False
                    for cl in range(NCL):
                        nc.vector.tensor_scalar_add(
                            o1T[e][:, 1 + cl * NCH:1 + (cl + 1) * NCH], pss[cl][:],
                            b1sb[:, e:e + 1],
                        )

                # ---- conv2 + residual + RMSNorm per chunk
                # all w2 loads for the half emitted first: keeps the Scalar
                # DMA queue free of compute-dependent ops ahead of them
                wb2s = []
                for cl in range(NCL):
                    per_cl = []
                    for dout in range(nD):
                        per_tap = []
                        for tap in (0, 1):
                            w = w2p.tile([128, nE, 128], f32r, tag="w2b", name="w2b")
                            (nc.scalar if tap == 0 else nc.gpsimd).dma_start(
                                out=w[:],
                                in_=w2pk.ap()[dout, tap].rearrange("e p m -> p e m"),
                            )
                            per_tap.append(w)
                        per_cl.append(per_tap)
                    wb2s.append(per_cl)
                for cl in range(NCL):
                    J0g = h * H + cl * NCH   # global output col
                    J0 = cl * NCH            # local o1T col
                    pssq = ps_ssq.tile([128, NCH], f32, tag="ps_ssq", name="ps_ssq")
                    ssqacc = sqp.tile([128, NCH], f32, tag="ssqacc", name="ssqacc")
                    ys = []
                    for dout in range(nD):
                        wb2 = wb2s[cl][dout]
                        py = ps_y.tile([128, NCH], f32, tag="ps_y", name="ps_y")
                        first = True
                        for e in range(nE):
                            for tap in (0, 1):
                                nc.tensor.matmul(
                                    py[:], wb2[tap][:, e, :],
                                    o1T[e][:, J0 + tap:J0 + tap + NCH],
                                    start=first, stop=(e == nE - 1 and tap == 1),
                                )
                                first = False
                        # y = (psum + b2) + x
                        yt = yp.tile([128, NCH], f32, tag=f"y_{dout}", name=f"y_{dout}")
                        k0 = cl * NCH + 2
                        nc.vector.scalar_tensor_tensor(
                            out=yt[:], in0=py[:], scalar=b2sb[:, dout:dout + 1],
                            in1=xh[dout][:, k0:k0 + NCH].bitcast(f32),
                            op0=ADD, op1=ADD,
                        )
                        ys.append(yt)
                        if dout == 0:
                            nc.vector.tensor_mul(ssqacc[:], yt[:], yt[:])
                        else:
                            ysq = ysqp.tile([128, NCH], f32, tag="ysq", name="ysq")
                            nc.vector.tensor_mul(ysq[:], yt[:], yt[:])
                            nc.vector.tensor_add(ssqacc[:], ssqacc[:], ysq[:])
                    # cross-partition sum, replicated to all partitions (f32 MM)
                    nc.tensor.matmul(pssq[:], ones[:], ssqacc[:],
                                     start=True, stop=True)
                    # inv_rms = 1/sqrt(mean + eps), replicated on all partitions
                    st = stp.tile([128, NCH], f32, tag="st", name="st")
                    nc.scalar.activation(
                        out=st[:], in_=pssq[:],
                        func=mybir.ActivationFunctionType.Sqrt,
                        bias=epssb[:], scale=1.0 / D_,
                    )
                    inv = invp.tile([128, NCH], f32, tag="inv", name="inv")
                    rscr = stp.tile([128, NCH], f32, tag="rscr", name="rscr")
                    nc.vector.reciprocal_approx_accurate(inv[:], st[:], rscr[:])
                    for dout in range(nD):
                        nc.vector.scalar_tensor_tensor(
                            out=ys[dout][:], in0=ys[dout][:],
                            scalar=lnwsb[:, dout:dout + 1],
                            in1=inv[:], op0=MUL, op1=MUL,
                        )
                        # spread store issues across idle DMA queues; Sync
                        # only for h1 (its queue holds h1 xh loads that wait
                        # on h0's residual reads - HOL hazard for h0 stores)
                        if h == 0:
                            st_eng = nc.gpsimd if dout % 2 else nc.scalar
                        else:
                            st_eng = (nc.scalar, nc.gpsimd, nc.sync)[dout % 3]
                        st_eng.dma_start(
                            out=outT.ap()[dout * 128:(dout + 1) * 128, J0g:J0g + NCH],
                            in_=ys[dout][:],
                        )

                if h == 0:
                    # chain the half boundary: o1(t0+H-1) -> col 0 for half 1
                    for e in range(nE):
                        nc.vector.tensor_copy(o1T[e][:, 0:1], o1T[e][:, H:H + 1])

            # last conv1 state (token t0+T-1) for the lf2 cache output
            for e in range(nE):
                nc.scalar.dma_start(
                    out=o1last.ap().rearrange("(e p) -> p e", p=128)[:, e:e + 1],
                    in_=o1T[e][:, H:H + 1].bitcast(f32),
                )

    nc.compile()
    return nc


# ------------------------------------------------------------------ host glue


def prepare_core_inputs(x3, lf1_cache, lf2_cache, w1, b1, w2, b2, ln_w,
                        ncores, S_, D_, DH_):
    """Build per-core in_maps. x3: [B, S, D] float32."""
    nD = D_ // 128
    nE = DH_ // 128
    B_ = x3.shape[0]
    T_ = (B_ * S_) // ncores
    per_seq = S_ // T_  # cores per sequence

    # lhsT tile-major packs (see build_module comments)
    w1pk = np.ascontiguousarray(
        w1.reshape(nE, 128, nD, 128, 2).transpose(0, 4, 2, 3, 1).astype(np.float32)
    )
    w2pk = np.ascontiguousarray(
        w2.reshape(nD, 128, nE, 128, 2).transpose(0, 4, 2, 3, 1).astype(np.float32)
    )
    b1c = np.ascontiguousarray(b1, np.float32)
    b2c = np.ascontiguousarray(b2, np.float32)
    lnc = np.ascontiguousarray(ln_w, np.float32)

    in_maps = []
    for c in range(ncores):
        b = c // per_seq
        part = c % per_seq
        t0 = part * T_
        x_ext = np.empty((T_ + 2, D_), np.float32)
        if part == 0:
            x_ext[0] = 0.0
            x_ext[1] = lf1_cache[b, :, 0, 0]
            o1_prev = lf2_cache[b, :, 0, 0]
        else:
            x_ext[0] = x3[b, t0 - 2]
            x_ext[1] = x3[b, t0 - 1]
            # single-token conv1 for the shard-boundary o1 state
            o1_prev = (w1[:, :, 0].astype(np.float32) @ x_ext[0]
                       + w1[:, :, 1].astype(np.float32) @ x_ext[1]
                       + b1.astype(np.float32))
        aux = np.ascontiguousarray(o1_prev.reshape(nE, 128).T.astype(np.float32))
        x_ext[2:] = x3[b, t0:t0 + T_]
        xT = np.ascontiguousarray(x_ext.T)
        in_maps.append({
            "xT": xT, "w1pk": w1pk, "w2pk": w2pk,
            "b1v": b1c, "b2v": b2c, "lnwv": lnc, "aux": aux,
        })
    return in_maps


_CACHE = {}


def _get_module():
    key = (D, DH, T)
    if key not in _CACHE:
        _CACHE[key] = build_module(D, DH, T, 512)
    return _CACHE[key]


def kernel(inputs, lf1_cache, lf2_cache, w1, b1, w2, b2, ln_w):
    from concourse.bass_utils import run_bass_kernel_spmd

    x = np.asarray(inputs, np.float32)
    lf1_cache = np.asarray(lf1_cache, np.float32)
    lf2_cache = np.asarray(lf2_cache, np.float32)
    w1 = np.asarray(w1, np.float32)
    b1 = np.asarray(b1, np.float32)
    w2 = np.asarray(w2, np.float32)
    b2 = np.asarray(b2, np.float32)
    ln_w = np.asarray(ln_w, np.float32)

    x3 = x.reshape(B, S, D)
    in_maps = prepare_core_inputs(x3, lf1_cache, lf2_cache, w1, b1, w2, b2,
                                  ln_w, NCORES, S, D, DH)
    nc = _get_module()
    res = run_bass_kernel_spmd(nc, in_maps, core_ids=list(range(NCORES)))

    per_seq = S // T
    lf_output = np.empty((B, S, D), np.float32)
    lf2 = np.empty((B, DH, 1, 1), np.float32)
    for c in range(NCORES):
        b = c // per_seq
        part = c % per_seq
        t0 = part * T
        lf_output[b, t0:t0 + T] = res.results[c]["outT"].T
        if part == per_seq - 1:
            lf2[b, :, 0, 0] = res.results[c]["o1last"]
    lf1 = np.ascontiguousarray(x3[:, -1][:, :, None, None])
    return lf_output, lf1, lf2


# revision 19
# speedup vs baseline: 1.0659x; 1.0022x over previous
"""LocalizedFiltering (conv1->conv2->residual->RMSNorm) TRN2 Bass kernel.

Full inputs in, full outputs out. Internally: data-parallel over 8 NeuronCores,
2048 tokens per core (each of the 4 sequences of 4096 tokens is split in half;
even cores take sequence starts, odd cores the second halves).

Device layout is channel-major (tokens on the free dim), so matmul contraction
(channels) sits on partitions for both operands. Host transposes per-core input
slabs and output slabs. Matmuls run as float32r (TF32-class rounding, full PE
rate at N=512). The causal kernel-size-2 convs need one previous token (x) and
one previous conv1 output (o1) per shard: previous-x rows come in via the input
slab; previous-o1 is computed on device from the two extra x rows (N=2 matmuls
folded into the conv1 weight loop) and blended against the lf2 cache with a
per-core 0/1 scalar so sequence-start cores use the cache instead. RMSNorm's
cross-partition sum uses a ones-matmul (bf16) accumulating into PSUM, which
also replicates the per-token sum across all partitions for the final scale.
"""

import numpy as np
from contextlib import ExitStack

NCORES = 8
B, S, D = 4, 4096, 2048
DH = D // 2
T = (B * S) // NCORES  # tokens per core
EPS = 1e-6


# ---------------------------------------------------------------- device code


def build_module(D_, DH_, T_, NCH, EPS_CONST=EPS):
    """Build + compile the per-core Bass module. All dims in channel units;
    NCH = token chunk width for matmuls (free dim)."""
    import concourse.tile as tile
    from concourse import bacc, mybir

    f32 = mybir.dt.float32
    f32r = mybir.dt.float32r
    bf16 = mybir.dt.bfloat16
    ADD = mybir.AluOpType.add
    MUL = mybir.AluOpType.mult

    nD = D_ // 128   # input-channel tiles (16)
    nE = DH_ // 128  # hidden-channel tiles (8)
    H = T_ // 2      # half size (1024)
    NCL = H // NCH   # chunks per half (2)

    nc = bacc.Bacc("TRN2", target_bir_lowering=False, debug=False)

    xT = nc.dram_tensor("xT", [D_, T_ + 2], f32r, kind="ExternalInput")
    # weights pre-packed on host, lhsT tile-major:
    # w1pk[e, tap, d, p, m] = w1[e*128+m, d*128+p, tap]
    w1pk = nc.dram_tensor("w1pk", [nE, 2, nD, 128, 128], f32r, kind="ExternalInput")
    # w2pk[do, tap, e, p, m] = w2[do*128+m, e*128+p, tap]
    w2pk = nc.dram_tensor("w2pk", [nD, 2, nE, 128, 128], f32r, kind="ExternalInput")
    b1v = nc.dram_tensor("b1v", [DH_], f32, kind="ExternalInput")
    b2v = nc.dram_tensor("b2v", [D_], f32, kind="ExternalInput")
    lnwv = nc.dram_tensor("lnwv", [D_], f32, kind="ExternalInput")
    # aux[:, e] = o1 state for token t0-1, hidden tile e (start cores: lf2
    # cache; mid cores: host-computed single-token conv1)
    aux = nc.dram_tensor("aux", [128, nE], f32, kind="ExternalInput")

    outT = nc.dram_tensor("outT", [D_, T_], f32, kind="ExternalOutput")
    o1last = nc.dram_tensor("o1last", [DH_], f32, kind="ExternalOutput")

    with tile.TileContext(nc) as tc:
        with ExitStack() as ctx:
            const = ctx.enter_context(tc.tile_pool(name="const", bufs=1))
            o1p = ctx.enter_context(tc.tile_pool(name="o1p", bufs=1))
            xhp = ctx.enter_context(tc.tile_pool(name="xhp", bufs=1))
            w1p = ctx.enter_context(tc.tile_pool(name="w1p", bufs=4))
            w2p = ctx.enter_context(tc.tile_pool(name="w2p", bufs=8))
            yp = ctx.enter_context(tc.tile_pool(name="yp", bufs=1))
            stp = ctx.enter_context(tc.tile_pool(name="stp", bufs=1))
            invp = ctx.enter_context(tc.tile_pool(name="invp", bufs=2))
            ysqp = ctx.enter_context(tc.tile_pool(name="ysqp", bufs=1))
            sqp = ctx.enter_context(tc.tile_pool(name="sqp", bufs=1))
            ps_o1 = ctx.enter_context(tc.tile_pool(name="ps_o1", bufs=2, space="PSUM"))
            ps_y = ctx.enter_context(tc.tile_pool(name="ps_y", bufs=5, space="PSUM"))
            ps_ssq = ctx.enter_context(tc.tile_pool(name="ps_ssq", bufs=1, space="PSUM"))

            # constants
            b1sb = const.tile([128, nE], f32, tag="b1sb")
            b2sb = const.tile([128, nD], f32, tag="b2sb")
            lnwsb = const.tile([128, nD], f32, tag="lnwsb")
            auxsb = const.tile([128, nE], f32, tag="auxsb")
            ones = const.tile([128, 128], f32, tag="ones")
            epssb = const.tile([128, 1], f32, tag="epssb")
            nc.vector.memset(epssb[:], EPS_CONST)
            nc.sync.dma_start(out=b1sb[:], in_=b1v.ap().rearrange("(e p) -> p e", p=128))
            nc.sync.dma_start(out=b2sb[:], in_=b2v.ap().rearrange("(e p) -> p e", p=128))
            nc.sync.dma_start(out=lnwsb[:], in_=lnwv.ap().rearrange("(e p) -> p e", p=128))
            nc.sync.dma_start(out=auxsb[:], in_=aux.ap())
            nc.vector.memset(ones[:], 1.0)

            # conv1 output for the current half, channel-major [DH_, H+1];
            # col 0 = previous token's o1 (blend for h0, chained from col H after)
            o1T = [o1p.tile([128, H + 1], f32r, tag=f"o1_{e}", name=f"o1_{e}")
                   for e in range(nE)]

            # previous-token o1 state into col 0 of each o1T tile
            for e in range(nE):
                nc.vector.tensor_copy(o1T[e][:, 0:1], auxsb[:, e:e + 1])

            # ~3.4us of dummy matmuls so PE_HAM un-throttles before the DMAs
            # land and real matmuls begin (f32: 4 cycles/row)
            warm = ps_ssq.tile([128, NCH], f32, tag="ps_ssq", name="warm")
            wN = min(128, NCH)
            for i in range(24):
                nc.tensor.matmul(warm[:, 0:wN], ones[:], ones[:, 0:wN],
                                 start=(i == 0), stop=(i == 23))

            for h in range(2):
                if h == 1:
                    # keep PE busy + HAM warm across the half transition (the
                    # PE otherwise idles ~5-10us here waiting on xh reloads and
                    # re-throttles to 1.2GHz)
                    warm2 = ps_o1.tile([128, NCH], f32, tag="ps_o1", name="warm2")
                    for i in range(6):
                        nc.tensor.matmul(warm2[:, 0:wN], ones[:], ones[:, 0:wN],
                                         start=(i == 0), stop=(i == 5))

                # ---- load x half: cols [h*H, h*H + H + 2) of xT
                xh = []
                for d in range(nD):
                    t = xhp.tile([128, H + 2], f32r, tag=f"xh_{d}", name=f"xh_{d}")
                    nc.sync.dma_start(
                        out=t[:], in_=xT.ap()[d * 128:(d + 1) * 128, h * H: h * H + H + 2]
                    )
                    xh.append(t)

                # ---- conv1: o1T local cols [1 + cl*NCH, +NCH) per chunk
                for e in range(nE):
                    wb = []
                    for tap in (0, 1):
                        w = w1p.tile([128, nD, 128], f32r, tag="w1b", name="w1b")
                        nc.gpsimd.dma_start(
                            out=w[:], in_=w1pk.ap()[e, tap].rearrange("d p m -> p d m")
                        )
                        wb.append(w)
                    pss = [ps_o1.tile([128, NCH], f32, tag="ps_o1", name="ps_o1")
                           for _ in range(NCL)]
                    first = True
                    for d in range(nD):
                        for tap in (0, 1):
                            wt = wb[tap][:, d, :]
                            for cl in range(NCL):
                                k0 = cl * NCH + 1 + tap
                                nc.tensor.matmul(
                                    pss[cl][:], wt, xh[d][:, k0:k0 + NCH],
                                    start=first, stop=(d == nD - 1 and tap == 1),
                                )
                            first = False
                    for cl in range(NCL):
                        nc.vector.tensor_scalar_add(
                            o1T[e][:, 1 + cl * NCH:1 + (cl + 1) * NCH], pss[cl][:],
                            b1sb[:, e:e + 1],
                        )

                # ---- conv2 + residual + RMSNorm per chunk
                # all w2 loads for the half emitted first: keeps the Scalar
                # DMA queue free of compute-dependent ops ahead of them
                wb2s = []
                for cl in range(NCL):
                    per_cl = []
                    for dout in range(nD):
                        per_tap = []
                        for tap in (0, 1):
                            w = w2p.tile([128, nE, 128], f32r, tag="w2b", name="w2b")
                            (nc.scalar if tap == 0 else nc.gpsimd).dma_start(
                                out=w[:],
                                in_=w2pk.ap()[dout, tap].rearrange("e p m -> p e m"),
                            )
                            per_tap.append(w)
                        per_cl.append(per_tap)
                    wb2s.append(per_cl)
                for cl in range(NCL):
                    J0g = h * H + cl * NCH   # global output col
                    J0 = cl * NCH            # local o1T col
                    pssq = ps_ssq.tile([128, NCH], f32, tag="ps_ssq", name="ps_ssq")
                    ssqacc = sqp.tile([128, NCH], f32, tag="ssqacc", name="ssqacc")
                    ys = []
                    for dout in range(nD):
                        wb2 = wb2s[cl][dout]
                        py = ps_y.tile([128, NCH], f32, tag="ps_y", name="ps_y")
                        first = True
                        for e in range(nE):
                            for tap in (0, 1):
                                nc.tensor.matmul(
                                    py[:], wb2[tap][:, e, :],
                                    o1T[e][:, J0 + tap:J0 + tap + NCH],
                                    start=first, stop=(e == nE - 1 and tap == 1),
                                )
                                first = False
                        # y = (psum + b2) + x
                        yt = yp.tile([128, NCH], f32, tag=f"y_{dout}", name=f"y_{dout}")
                        k0 = cl * NCH + 2
                        nc.vector.scalar_tensor_tensor(
                            out=yt[:], in0=py[:], scalar=b2sb[:, dout:dout + 1],
                            in1=xh[dout][:, k0:k0 + NCH].bitcast(f32),
                            op0=ADD, op1=ADD,
                        )
                        ys.append(yt)
                        if dout == 0:
                            nc.vector.tensor_mul(ssqacc[:], yt[:], yt[:])
                        else:
                            ysq = ysqp.tile([128, NCH], f32, tag="ysq", name="ysq")
                            nc.vector.tensor_mul(ysq[:], yt[:], yt[:])
                            nc.vector.tensor_add(ssqacc[:], ssqacc[:], ysq[:])
                    # cross-partition sum, replicated to all partitions (f32 MM)
                    nc.tensor.matmul(pssq[:], ones[:], ssqacc[:],
                                     start=True, stop=True)
                    # inv_rms = 1/sqrt(mean + eps), replicated on all partitions
                    st = stp.tile([128, NCH], f32, tag="st", name="st")
                    nc.scalar.activation(
                        out=st[:], in_=pssq[:],
                        func=mybir.ActivationFunctionType.Sqrt,
                        bias=epssb[:], scale=1.0 / D_,
                    )
                    inv = invp.tile([128, NCH], f32, tag="inv", name="inv")
                    rscr = stp.tile([128, NCH], f32, tag="rscr", name="rscr")
                    nc.vector.reciprocal_approx_accurate(inv[:], st[:], rscr[:])
                    for dout in range(nD):
                        nc.vector.scalar_tensor_tensor(
                            out=ys[dout][:], in0=ys[dout][:],
                            scalar=lnwsb[:, dout:dout + 1],
                            in1=inv[:], op0=MUL, op1=MUL,
                        )
                        # spread store issues across idle DMA queues; Sync
                        # only for h1 (its queue holds h1 xh loads that wait
                        # on h0's residual reads - HOL hazard for h0 stores)
                        if h == 0:
                            # all h0 stores on Scalar: GpSimd must stay free
                            # of late-waiting ops so h1 w1 prefetch can fire
                            st_eng = nc.scalar
                        else:
                            st_eng = (nc.scalar, nc.gpsimd, nc.sync)[dout % 3]
                        st_eng.dma_start(
                            out=outT.ap()[dout * 128:(dout + 1) * 128, J0g:J0g + NCH],
                            in_=ys[dout][:],
                        )

                if h == 0:
                    # chain the half boundary: o1(t0+H-1) -> col 0 for half 1
                    for e in range(nE):
                        nc.vector.tensor_copy(o1T[e][:, 0:1], o1T[e][:, H:H + 1])

            # last conv1 state (token t0+T-1) for the lf2 cache output
            for e in range(nE):
                nc.scalar.dma_start(
                    out=o1last.ap().rearrange("(e p) -> p e", p=128)[:, e:e + 1],
                    in_=o1T[e][:, H:H + 1].bitcast(f32),
                )

    nc.compile()
    return nc


# ------------------------------------------------------------------ host glue


def prepare_core_inputs(x3, lf1_cache, lf2_cache, w1, b1, w2, b2, ln_w,
                        ncores, S_, D_, DH_):
    """Build per-core in_maps. x3: [B, S, D] float32."""
    nD = D_ // 128
    nE = DH_ // 128
    B_ = x3.shape[0]
    T_ = (B_ * S_) // ncores
    per_seq = S_ // T_  # cores per sequence

    # lhsT tile-major packs (see build_module comments)
    w1pk = np.ascontiguousarray(
        w1.reshape(nE, 128, nD, 128, 2).transpose(0, 4, 2, 3, 1).astype(np.float32)
    )
    w2pk = np.ascontiguousarray(
        w2.reshape(nD, 128, nE, 128, 2).transpose(0, 4, 2, 3, 1).astype(np.float32)
    )
    b1c = np.ascontiguousarray(b1, np.float32)
    b2c = np.ascontiguousarray(b2, np.float32)
    lnc = np.ascontiguousarray(ln_w, np.float32)

    in_maps = []
    for c in range(ncores):
        b = c // per_seq
        part = c % per_seq
        t0 = part * T_
        x_ext = np.empty((T_ + 2, D_), np.float32)
        if part == 0:
            x_ext[0] = 0.0
            x_ext[1] = lf1_cache[b, :, 0, 0]
            o1_prev = lf2_cache[b, :, 0, 0]
        else:
            x_ext[0] = x3[b, t0 - 2]
            x_ext[1] = x3[b, t0 - 1]
            # single-token conv1 for the shard-boundary o1 state
            o1_prev = (w1[:, :, 0].astype(np.float32) @ x_ext[0]
                       + w1[:, :, 1].astype(np.float32) @ x_ext[1]
                       + b1.astype(np.float32))
        aux = np.ascontiguousarray(o1_prev.reshape(nE, 128).T.astype(np.float32))
        x_ext[2:] = x3[b, t0:t0 + T_]
        xT = np.ascontiguousarray(x_ext.T)
        in_maps.append({
            "xT": xT, "w1pk": w1pk, "w2pk": w2pk,
            "b1v": b1c, "b2v": b2c, "lnwv": lnc, "aux": aux,
        })
    return in_maps


_CACHE = {}


def _get_module():
    key = (D, DH, T)
    if key not in _CACHE:
        _CACHE[key] = build_module(D, DH, T, 512)
    return _CACHE[key]


def kernel(inputs, lf1_cache, lf2_cache, w1, b1, w2, b2, ln_w):
    from concourse.bass_utils import run_bass_kernel_spmd

    x = np.asarray(inputs, np.float32)
    lf1_cache = np.asarray(lf1_cache, np.float32)
    lf2_cache = np.asarray(lf2_cache, np.float32)
    w1 = np.asarray(w1, np.float32)
    b1 = np.asarray(b1, np.float32)
    w2 = np.asarray(w2, np.float32)
    b2 = np.asarray(b2, np.float32)
    ln_w = np.asarray(ln_w, np.float32)

    x3 = x.reshape(B, S, D)
    in_maps = prepare_core_inputs(x3, lf1_cache, lf2_cache, w1, b1, w2, b2,
                                  ln_w, NCORES, S, D, DH)
    nc = _get_module()
    res = run_bass_kernel_spmd(nc, in_maps, core_ids=list(range(NCORES)))

    per_seq = S // T
    lf_output = np.empty((B, S, D), np.float32)
    lf2 = np.empty((B, DH, 1, 1), np.float32)
    for c in range(NCORES):
        b = c // per_seq
        part = c % per_seq
        t0 = part * T
        lf_output[b, t0:t0 + T] = res.results[c]["outT"].T
        if part == per_seq - 1:
            lf2[b, :, 0, 0] = res.results[c]["o1last"]
    lf1 = np.ascontiguousarray(x3[:, -1][:, :, None, None])
    return lf_output, lf1, lf2
